# revision 12
# baseline (speedup 1.0000x reference)
"""Trainium2 Bass kernel for nn_ActorCritic (GIN actor-critic, 8 disjoint graphs).

Sharding: graph b -> NeuronCore b (data parallel over the batch of disjoint
graphs). Each core holds its diagonal adjacency block (transposed, bf16),
its node features, and replicated MLP weights. The only cross-core traffic
is the 4 BatchNorm statistics exchanges, done as tiny AllReduces.

Host side only reshapes / transposes / one-hot-encodes inputs into the exact
SBUF layouts (no model math on CPU), so every input DMA is contiguous.
"""

import numpy as np
import ml_dtypes

import concourse.bass as bass
import concourse.bacc as bacc
import concourse.mybir as mybir
import concourse.tile as tile
from concourse.bass_utils import run_bass_kernel_spmd

# ---- problem constants ----
B = 8            # graphs == cores
N = 2000         # nodes per graph
NJ = 100         # candidates per graph
HID = 64
HACT = 32
BN_EPS = 1e-5
CNT = float(B * N)          # batchnorm count (16000)
NEG_BIG = -1.0e30

NT = 16                      # node k-tiles of 128 (last has 80 rows)
K_LIST = [128] * 15 + [80]
CHUNKS = [(0, 512), (512, 512), (1024, 512), (1536, 464)]  # node columns

f32 = mybir.dt.float32
bf16 = mybir.dt.bfloat16

AX = mybir.AxisListType.X
ALU = mybir.AluOpType
ACT = mybir.ActivationFunctionType

# wpack column layout: [w2l1 64][w1l2 64][w2l2 64][gbe 8][eps 1][actw 96]
WP_W2L1 = 0
WP_W1L2 = 64
WP_W2L2 = 128
WP_GBE = 192
WP_EPS = 200
WP_ACT = 201
WP_COLS = 297

# consts tensor [128, CN_COLS] f32: w1l1 (rows 0-1), headw (rows 0-31),
# maskval (row 0), identity (rows 0-63)
CN_W1L1 = 0      # [2, 64]
CN_HEADW = 64    # [32, 4]
CN_MASK = 68     # [1, 100]
CN_IDENT = 168   # [64, 64]
CN_COLS = 232


def build_graph(bc2: float, reps: int = 1):
    nc = bacc.Bacc("TRN2", target_bir_lowering=False, debug=False,
                   num_devices=B)

    # all pre-shuffled on host to the exact SBUF layout (contiguous DMAs)
    adjT_e = nc.dram_tensor("adjT", [128, NT * N], bf16, kind="ExternalInput")
    xn_e = nc.dram_tensor("xn", [128, NT * 2], bf16, kind="ExternalInput")
    # paug + wpack packed into one tensor (wpack in rows 0-63 after paug cols)
    paug_e = nc.dram_tensor("paug", [128, NT * (NJ + 1) + WP_COLS], f32,
                            kind="ExternalInput")
    consts_e = nc.dram_tensor("consts", [128, CN_COLS], f32,
                              kind="ExternalInput")
    out_e = nc.dram_tensor("out", [1, NJ + 1], f32, kind="ExternalOutput")

    with tile.TileContext(nc) as tc:
        with (
            tc.tile_pool(name="sb", bufs=1) as sb,
            tc.tile_pool(name="ps", bufs=1, space="PSUM") as ps,
            tc.tile_pool(name="dr", bufs=1, space="DRAM") as dr,
        ):
            for rep in range(reps):
                _emit_one(nc, sb, ps, dr, adjT_e, xn_e, paug_e, consts_e,
                          out_e, bc2)
    nc.compile()
    return nc


def _emit_one(nc, sb, ps, dr, adjT_e, xn_e, paug_e, consts_e, out_e, bc2):
    # ---------------- input DMAs (all contiguous) ----------------
    adjT = sb.tile([128, NT * N], bf16, tag="adjT")
    nc.sync.dma_start(adjT[:, :], adjT_e[:, :])
    xn = sb.tile([128, NT * 2], bf16, tag="xn")
    nc.sync.dma_start(xn[:, :], xn_e[:, :])
    paug = sb.tile([128, NT * (NJ + 1) + WP_COLS], f32, tag="paug")
    nc.sync.dma_start(paug[:, :], paug_e[:, :])
    cn = sb.tile([128, CN_COLS], f32, tag="cn")
    nc.sync.dma_start(cn[:, :], consts_e[:, :])

    WP0 = NT * (NJ + 1)

    def wpc(a, b):          # wpack column slice [64, b-a]
        return paug[0:HID, WP0 + a:WP0 + b]

    def cnc(p, a, b):       # consts slice [p, b-a]
        return cn[0:p, a:b]

    sq = sb.tile([HID, 2048], f32, tag="sq")
    h1m = sb.tile([HID, N], f32, tag="h1m")
    h1 = sb.tile([HID, N], f32, tag="h1")
    h2m = sb.tile([HID, N], f32, tag="h2m")
    h2 = sb.tile([HID, N], f32, tag="h2")
    u2 = sb.tile([128, NT * HID], bf16, tag="u2")
    h2n = sb.tile([128, NT * HID], f32, tag="h2n")

    eps_ap = wpc(WP_EPS, WP_EPS + 1)

    # ---------------- batchnorm block ----------------
    def bn_block(i, zp, out_sb):
        """out_sb = relu((z - mean)*g*rsqrt(var+eps) + be), stats global.

        Stats on DVE, then one AllReduce of [64, 2] (sum, sumsq), then the
        scale/bias chain entirely on the scalar engine (no cross-engine sync).
        """
        stats2 = sb.tile([HID, 2], f32, tag=f"st2_{i}")
        nc.vector.reduce_sum(stats2[:, 0:1], zp[0:HID, 0:N], axis=AX)
        nc.scalar.activation(sq[:, 0:N], zp[0:HID, 0:N], ACT.Square,
                             accum_out=stats2[:, 1:2])
        cc_in = dr.tile([HID, 2], f32, tag=f"ccin_{i}")
        cc_out = dr.tile([HID, 2], f32, tag=f"ccout_{i}")
        nc.sync.dma_start(cc_in[:, :], stats2[:, :])
        nc.gpsimd.collective_compute(
            "AllReduce", ALU.add,
            replica_groups=[list(range(B))],
            ins=[cc_in.opt()], outs=[cc_out.opt()])
        g_sb = sb.tile([HID, 2], f32, tag=f"g_{i}")
        nc.sync.dma_start(g_sb[:, :], cc_out[:, :])
        w = sb.tile([HID, 8], f32, tag=f"bnv_{i}")
        S, Q = g_sb[:, 0:1], g_sb[:, 1:2]
        # chain on scalar engine: out = func(in*scale + bias)
        nc.scalar.activation(w[:, 0:1], S, ACT.Square, scale=1.0 / CNT)  # m^2
        nc.scalar.activation(w[:, 1:2], Q, ACT.Identity, scale=1.0 / CNT,
                             bias=eps_ap)                      # Q/CNT + eps
        nc.scalar.activation(w[:, 2:3], w[:, 0:1], ACT.Identity, scale=-1.0,
                             bias=w[:, 1:2])                   # var + eps
        nc.scalar.activation(w[:, 3:4], w[:, 2:3],
                             ACT.Abs_reciprocal_sqrt)          # rsqrt(var+eps)
        g_col = wpc(WP_GBE + 2 * i, WP_GBE + 2 * i + 1)
        be_col = wpc(WP_GBE + 2 * i + 1, WP_GBE + 2 * i + 2)
        nc.scalar.activation(w[:, 4:5], w[:, 3:4], ACT.Identity,
                             scale=g_col)                      # scale = g*rsq
        nc.scalar.activation(w[:, 5:6], S, ACT.Identity,
                             scale=w[:, 4:5])                  # S*scale
        nc.scalar.activation(w[:, 6:7], w[:, 5:6], ACT.Identity,
                             scale=-1.0 / CNT, bias=be_col)    # bias
        nc.scalar.activation(out_sb[:, 0:N], zp[0:HID, 0:N], ACT.Relu,
                             bias=w[:, 6:7], scale=w[:, 4:5])

    # ---------------- layer 1: pooled = adj @ x ----------------
    pb = ps.tile([128, 2048], f32, tag="pb")
    for k in range(NT):
        kk = K_LIST[k]
        for (c0, ln) in CHUNKS:
            nc.tensor.matmul(pb[0:2, c0:c0 + ln],
                             lhsT=xn[0:kk, 2 * k:2 * k + 2],
                             rhs=adjT[0:kk, N * k + c0:N * k + c0 + ln],
                             start=(k == 0), stop=(k == NT - 1),
                             skip_group_check=True)
    pooled = sb.tile([2, N], f32, tag="pooled")
    nc.vector.tensor_copy(pooled[:, :], pb[0:2, 0:N])
    # rep1 = pooled^T W1  (channel-major [64, N])
    pa = ps.tile([128, 2048], f32, tag="pa")
    for (c0, ln) in CHUNKS:
        nc.tensor.matmul(pa[0:HID, c0:c0 + ln], lhsT=cnc(2, CN_W1L1, CN_W1L1 + HID),
                         rhs=pooled[:, c0:c0 + ln], start=True, stop=True)
    bn_block(0, pa, h1m)

    # ---------------- rep2 = h1m @ W2l1 ----------------
    pb = ps.tile([128, 2048], f32, tag="pb")
    for (c0, ln) in CHUNKS:
        nc.tensor.matmul(pb[0:HID, c0:c0 + ln],
                         lhsT=wpc(WP_W2L1, WP_W2L1 + HID),
                         rhs=h1m[:, c0:c0 + ln], start=True, stop=True)
    bn_block(1, pb, h1)

    # ---------------- u2 = h1 @ W1l2 (node-major, bf16) ----------------
    pa = ps.tile([128, 2048], f32, tag="pa")
    for k in range(NT):
        kk = K_LIST[k]
        nc.tensor.matmul(pa[0:kk, HID * k:HID * (k + 1)],
                         lhsT=h1[0:HID, 128 * k:128 * k + kk],
                         rhs=wpc(WP_W1L2, WP_W1L2 + HID),
                         start=True, stop=True)
    nc.vector.tensor_copy(u2[:, :], pa[0:128, 0:NT * HID])

    # ---------------- layer 2: z2 = adj @ u2 ----------------
    pb = ps.tile([128, 2048], f32, tag="pb")
    for k in range(NT):
        kk = K_LIST[k]
        for (c0, ln) in CHUNKS:
            nc.tensor.matmul(pb[0:HID, c0:c0 + ln],
                             lhsT=u2[0:kk, HID * k:HID * (k + 1)],
                             rhs=adjT[0:kk, N * k + c0:N * k + c0 + ln],
                             start=(k == 0), stop=(k == NT - 1),
                             skip_group_check=True)
    bn_block(2, pb, h2m)

    # ---------------- rep2l2 = h2m @ W2l2 ----------------
    pa = ps.tile([128, 2048], f32, tag="pa")
    for (c0, ln) in CHUNKS:
        nc.tensor.matmul(pa[0:HID, c0:c0 + ln],
                         lhsT=wpc(WP_W2L2, WP_W2L2 + HID),
                         rhs=h2m[:, c0:c0 + ln], start=True, stop=True)
    bn_block(3, pa, h2)

    # ---------------- transpose h2 -> node-major ----------------
    pb = ps.tile([128, 2048], f32, tag="pb")
    for k in range(NT):
        kk = K_LIST[k]
        nc.tensor.transpose(pb[0:kk, HID * k:HID * (k + 1)],
                            h2[0:HID, 128 * k:128 * k + kk],
                            cnc(HID, CN_IDENT, CN_IDENT + HID))
    nc.vector.tensor_copy(h2n[:, :], pb[0:128, 0:NT * HID])

    # ---------------- candidates + pooling: C = h2n^T @ paug ----------------
    pa = ps.tile([128, 2048], f32, tag="pa")
    for k in range(NT):
        kk = K_LIST[k]
        nc.tensor.matmul(pa[0:HID, 0:NJ + 1],
                         lhsT=h2n[0:kk, HID * k:HID * (k + 1)],
                         rhs=paug[0:kk, (NJ + 1) * k:(NJ + 1) * (k + 1)],
                         start=(k == 0), stop=(k == NT - 1))
    C = sb.tile([HID, NJ + 1], f32, tag="C")
    nc.vector.tensor_copy(C[:, :], pa[0:HID, 0:NJ + 1])

    # ---------------- actor / critic heads ----------------
    pb2 = ps.tile([128, 2048], f32, tag="pb")
    nc.tensor.matmul(pb2[0:HACT, 0:NJ + 1],
                     lhsT=wpc(WP_ACT, WP_ACT + HACT),
                     rhs=C[:, :], start=True, stop=True)
    nc.tensor.matmul(pb2[0:2 * HACT, 512:513],
                     lhsT=wpc(WP_ACT + HACT, WP_ACT + 3 * HACT),
                     rhs=C[:, NJ:NJ + 1], start=True, stop=True)
    hw = sb.tile([HACT, 2], f32, tag="hw")
    nc.vector.tensor_add(hw[:, 0:1], pb2[0:HACT, 512:513],
                         cnc(HACT, CN_HEADW + 2, CN_HEADW + 3))
    T = sb.tile([HACT, NJ + 1], f32, tag="T")
    nc.scalar.activation(T[:, 0:NJ], pb2[0:HACT, 0:NJ], ACT.Tanh,
                         bias=hw[:, 0:1])
    nc.scalar.activation(T[:, NJ:NJ + 1], pb2[HACT:2 * HACT, 512:513],
                         ACT.Tanh,
                         bias=cnc(HACT, CN_HEADW + 3, CN_HEADW + 4))
    nc.tensor.matmul(pb2[0:1, 1024:1024 + NJ], lhsT=cnc(HACT, CN_HEADW, CN_HEADW + 1),
                     rhs=T[:, 0:NJ], start=True, stop=True)
    nc.tensor.matmul(pb2[0:1, 1536:1537], lhsT=cnc(HACT, CN_HEADW + 1, CN_HEADW + 2),
                     rhs=T[:, NJ:NJ + 1], start=True, stop=True)
    pit = sb.tile([1, NJ + 1], f32, tag="pit")
    sm = sb.tile([1, NJ], f32, tag="sm")
    # scores are tanh-bounded (|s| < ~6), so exp() without max-subtraction
    # is safe; masked entries carry -1e30 -> exp underflows to exactly 0.
    nc.vector.tensor_add(sm[:, :], pb2[0:1, 1024:1024 + NJ],
                         cnc(1, CN_MASK, CN_MASK + NJ))
    red = sb.tile([1, 2], f32, tag="red")
    e = sb.tile([1, NJ], f32, tag="e")
    nc.scalar.activation(e[:, :], sm[:, :], ACT.Exp, accum_out=red[:, 0:1])
    nc.vector.reciprocal(red[:, 1:2], red[:, 0:1])
    nc.vector.tensor_scalar_mul(pit[:, 0:NJ], e[:, :], red[:, 1:2])
    nc.scalar.activation(pit[:, NJ:NJ + 1], pb2[0:1, 1536:1537],
                         ACT.Copy, bias=float(bc2))
    nc.sync.dma_start(out_e[:, :], pit[:, :])


# ---------------- host side ----------------

def _shuffle_tiles(arr2d, cols):
    """[2000+, cols] node-major -> [128, NT*cols] SBUF tile layout."""
    out = np.zeros((128, NT * cols), dtype=arr2d.dtype)
    for t in range(NT):
        rows = arr2d[128 * t:128 * (t + 1)]
        out[0:rows.shape[0], cols * t:cols * t + cols] = rows
    return out


def _prep_in_maps(x, graph_pool, adj, candidate, mask, params):
    x = np.asarray(x, dtype=np.float32)
    graph_pool = np.asarray(graph_pool, dtype=np.float32)
    adj = np.asarray(adj, dtype=np.float32)
    candidate = np.asarray(candidate).astype(np.int64)
    mask = np.asarray(mask)
    g0, g1 = params['gin'][0], params['gin'][1]
    a, c = params['actor'], params['critic']
    f = lambda t: np.ascontiguousarray(np.asarray(t, dtype=np.float32))

    w1l1 = f(g0['W1'])
    gbe = np.stack([f(g0['g1']), f(g0['be1']), f(g0['g']), f(g0['be']),
                    f(g1['g1']), f(g1['be1']), f(g1['g']), f(g1['be'])],
                   axis=1)  # [64, 8]
    wa1 = f(a['W1'])                       # [128, 32]
    wpack = np.concatenate(
        [f(g0['W2']), f(g1['W1']), f(g1['W2']), gbe,
         np.full((HID, 1), BN_EPS, np.float32),
         wa1[:HID], wa1[HID:], f(c['W1'])], axis=1)
    assert wpack.shape == (HID, WP_COLS)
    headw = np.stack([f(a['W2'])[:, 0], f(c['W2'])[:, 0],
                      np.broadcast_to(f(a['b1']), (HACT,)),
                      np.broadcast_to(f(c['b1']), (HACT,))], axis=1)
    ba2 = float(np.asarray(a['b2']).reshape(-1)[0])
    bc2 = float(np.asarray(c['b2']).reshape(-1)[0])

    in_maps = []
    for b in range(B):
        sl = slice(b * N, (b + 1) * N)
        adjT = np.ascontiguousarray(adj[sl, sl].T)
        adjT = _shuffle_tiles(adjT, N).astype(ml_dtypes.bfloat16)
        xn = _shuffle_tiles(x[sl], 2).astype(ml_dtypes.bfloat16)
        ph = np.zeros((N, NJ + 1), dtype=np.float32)
        ph[candidate[b], np.arange(NJ)] = 1.0
        ph[:, NJ] = graph_pool[b, sl]
        paug = np.zeros((128, NT * (NJ + 1) + WP_COLS), dtype=np.float32)
        paug[:, :NT * (NJ + 1)] = _shuffle_tiles(ph, NJ + 1)
        paug[0:HID, NT * (NJ + 1):] = wpack
        maskval = (np.where(mask[b], NEG_BIG, 0.0) + ba2) \
            .astype(np.float32).reshape(1, NJ)
        consts = np.zeros((128, CN_COLS), dtype=np.float32)
        consts[0:2, CN_W1L1:CN_W1L1 + HID] = w1l1
        consts[0:HACT, CN_HEADW:CN_HEADW + 4] = headw
        consts[0:1, CN_MASK:CN_MASK + NJ] = maskval
        consts[0:HID, CN_IDENT:CN_IDENT + HID] = np.eye(HID, dtype=np.float32)
        in_maps.append({
            "adjT": adjT, "xn": xn, "paug": paug, "consts": consts,
        })
    return in_maps, bc2


def run(inputs: dict, reps: int = 1, nc=None):
    """Run on hardware; returns ((pi, v), nc) so callers can reuse the graph."""
    in_maps, bc2 = _prep_in_maps(
        inputs['x'], inputs['graph_pool'], inputs['adj'],
        inputs['candidate'], inputs['mask'], inputs['params'])
    if nc is None:
        nc = build_graph(bc2, reps=reps)
    res = run_bass_kernel_spmd(nc, in_maps, core_ids=list(range(B)))
    outs = [res.results[i]["out"] for i in range(B)]
    pi = np.stack([o[0, :NJ] for o in outs]).astype(np.float32)[:, :, None]
    v = np.stack([o[0, NJ:NJ + 1] for o in outs]).astype(np.float32)
    return (pi, v), nc


def kernel(x, graph_pool, padded_nei, adj, candidate, mask, params):
    (pi, v), _ = run({'x': x, 'graph_pool': graph_pool, 'adj': adj,
                      'candidate': candidate, 'mask': mask, 'params': params})
    return pi, v


# revision 13
# speedup vs baseline: 1.1671x; 1.1671x over previous
"""Trainium2 Bass kernel for nn_ActorCritic (GIN actor-critic, 8 disjoint graphs).

Sharding: graph b -> NeuronCore b (data parallel over the batch of disjoint
graphs). Each core holds its diagonal adjacency block (transposed, bf16),
its node features, and replicated MLP weights. The only cross-core traffic
is the 4 BatchNorm statistics exchanges, done as tiny AllReduces.

Host side only reshapes / transposes / one-hot-encodes inputs into the exact
SBUF layouts (no model math on CPU), so every input DMA is contiguous.
"""

import numpy as np
import ml_dtypes

import concourse.bass as bass
import concourse.bacc as bacc
import concourse.mybir as mybir
import concourse.tile as tile
from concourse.bass_utils import run_bass_kernel_spmd

# ---- problem constants ----
B = 8            # graphs == cores
N = 2000         # nodes per graph
NJ = 100         # candidates per graph
HID = 64
HACT = 32
BN_EPS = 1e-5
CNT = float(B * N)          # batchnorm count (16000)
NEG_BIG = -1.0e30

NT = 16                      # node k-tiles of 128 (last has 80 rows)
K_LIST = [128] * 15 + [80]
CHUNKS = [(0, 512), (512, 512), (1024, 512), (1536, 464)]  # node columns

f32 = mybir.dt.float32
bf16 = mybir.dt.bfloat16

AX = mybir.AxisListType.X
ALU = mybir.AluOpType
ACT = mybir.ActivationFunctionType

# wpack column layout: [w2l1 64][w1l2 64][w2l2 64][gbe 8][eps 1][actw 96]
WP_W2L1 = 0
WP_W1L2 = 64
WP_W2L2 = 128
WP_GBE = 192
WP_EPS = 200
WP_ACT = 201
WP_COLS = 297

# consts tensor [128, CN_COLS] f32: w1l1 (rows 0-1), headw (rows 0-31),
# maskval (row 0), identity (rows 0-63)
CN_W1L1 = 0      # [2, 64]
CN_HEADW = 64    # [32, 4]
CN_MASK = 68     # [1, 100]
CN_IDENT = 168   # [64, 64]
CN_ZERO = 232    # always-zero column (bias for non-Copy activations)
CN_COLS = 236


def build_graph(bc2: float, reps: int = 1):
    nc = bacc.Bacc("TRN2", target_bir_lowering=False, debug=False,
                   num_devices=B)

    # all pre-shuffled on host to the exact SBUF layout (contiguous DMAs)
    adjT_e = nc.dram_tensor("adjT", [128, NT * N], bf16, kind="ExternalInput")
    xn_e = nc.dram_tensor("xn", [128, NT * 2], bf16, kind="ExternalInput")
    # paug + wpack packed into one tensor (wpack in rows 0-63 after paug cols)
    paug_e = nc.dram_tensor("paug", [128, NT * (NJ + 1) + WP_COLS], f32,
                            kind="ExternalInput")
    consts_e = nc.dram_tensor("consts", [128, CN_COLS], f32,
                              kind="ExternalInput")
    out_e = nc.dram_tensor("out", [1, NJ + 1], f32, kind="ExternalOutput")

    with tile.TileContext(nc) as tc:
        with (
            tc.tile_pool(name="sb", bufs=1) as sb,
            tc.tile_pool(name="ps", bufs=1, space="PSUM") as ps,
            tc.tile_pool(name="dr", bufs=1, space="DRAM") as dr,
        ):
            for rep in range(reps):
                _emit_one(nc, sb, ps, dr, adjT_e, xn_e, paug_e, consts_e,
                          out_e, bc2)
    nc.compile()
    return nc


def _emit_one(nc, sb, ps, dr, adjT_e, xn_e, paug_e, consts_e, out_e, bc2):
    # ---------------- input DMAs (all contiguous) ----------------
    adjT = sb.tile([128, NT * N], bf16, tag="adjT")
    nc.sync.dma_start(adjT[:, :], adjT_e[:, :])
    xn = sb.tile([128, NT * 2], bf16, tag="xn")
    nc.sync.dma_start(xn[:, :], xn_e[:, :])
    paug = sb.tile([128, NT * (NJ + 1) + WP_COLS], f32, tag="paug")
    nc.sync.dma_start(paug[:, :], paug_e[:, :])
    cn = sb.tile([128, CN_COLS], f32, tag="cn")
    nc.sync.dma_start(cn[:, :], consts_e[:, :])

    WP0 = NT * (NJ + 1)

    def wpc(a, b):          # wpack column slice [64, b-a]
        return paug[0:HID, WP0 + a:WP0 + b]

    def cnc(p, a, b):       # consts slice [p, b-a]
        return cn[0:p, a:b]

    sq = sb.tile([HID, 2048], f32, tag="sq")
    h1m = sb.tile([HID, N], f32, tag="h1m")
    h1 = sb.tile([HID, N], f32, tag="h1")
    h2m = sb.tile([HID, N], f32, tag="h2m")
    h2 = sb.tile([HID, N], f32, tag="h2")
    u2 = sb.tile([128, NT * HID], bf16, tag="u2")
    h2n = sb.tile([128, NT * HID], f32, tag="h2n")

    eps_ap = wpc(WP_EPS, WP_EPS + 1)

    # ---------------- batchnorm block ----------------
    def bn_block(i, zp, out_sb):
        """out_sb = relu((z - mean)*g*rsqrt(var+eps) + be), stats global.

        Stats on DVE, then one AllReduce of [64, 2] (sum, sumsq), then the
        scale/bias chain entirely on the scalar engine (no cross-engine sync).
        """
        zb = cnc(HID, CN_ZERO, CN_ZERO + 1)
        stats2 = sb.tile([HID, 2], f32, tag="st2")
        nc.scalar.activation(sq[:, 0:N], zp[0:HID, 0:N], ACT.Copy,
                             accum_out=stats2[:, 0:1])
        nc.scalar.activation(sq[:, 0:N], zp[0:HID, 0:N], ACT.Square,
                             bias=zb, accum_out=stats2[:, 1:2])
        cc_in = dr.tile([HID, 2], f32, tag="ccin")
        cc_out = dr.tile([HID, 2], f32, tag="ccout")
        nc.sync.dma_start(cc_in[:, :], stats2[:, :])
        nc.gpsimd.collective_compute(
            "AllReduce", ALU.add,
            replica_groups=[list(range(B))],
            ins=[cc_in.opt()], outs=[cc_out.opt()])
        g_sb = sb.tile([HID, 2], f32, tag="gsb")
        nc.sync.dma_start(g_sb[:, :], cc_out[:, :])
        w = sb.tile([HID, 8], f32, tag="bnv")
        S, Q = g_sb[:, 0:1], g_sb[:, 1:2]
        # chain on scalar engine: out = func(in*scale + bias)
        nc.scalar.activation(w[:, 0:1], S, ACT.Square, scale=1.0 / CNT,
                             bias=zb)                          # m^2
        nc.scalar.activation(w[:, 1:2], Q, ACT.Identity, scale=1.0 / CNT,
                             bias=eps_ap)                      # Q/CNT + eps
        nc.scalar.activation(w[:, 2:3], w[:, 0:1], ACT.Identity, scale=-1.0,
                             bias=w[:, 1:2])                   # var + eps
        nc.scalar.activation(w[:, 3:4], w[:, 2:3],
                             ACT.Abs_reciprocal_sqrt, bias=zb) # rsqrt(var+eps)
        g_col = wpc(WP_GBE + 2 * i, WP_GBE + 2 * i + 1)
        be_col = wpc(WP_GBE + 2 * i + 1, WP_GBE + 2 * i + 2)
        nc.scalar.activation(w[:, 4:5], w[:, 3:4], ACT.Identity,
                             scale=g_col, bias=zb)             # scale = g*rsq
        nc.scalar.activation(w[:, 5:6], S, ACT.Identity,
                             scale=w[:, 4:5], bias=zb)         # S*scale
        nc.scalar.activation(w[:, 6:7], w[:, 5:6], ACT.Identity,
                             scale=-1.0 / CNT, bias=be_col)    # bias
        nc.scalar.activation(out_sb[:, 0:N], zp[0:HID, 0:N], ACT.Relu,
                             bias=w[:, 6:7], scale=w[:, 4:5])

    # ---------------- layer 1: pooled = adj @ x ----------------
    pb = ps.tile([128, 2048], f32, tag="pb")
    for k in range(NT):
        kk = K_LIST[k]
        for (c0, ln) in CHUNKS:
            nc.tensor.matmul(pb[0:2, c0:c0 + ln],
                             lhsT=xn[0:kk, 2 * k:2 * k + 2],
                             rhs=adjT[0:kk, N * k + c0:N * k + c0 + ln],
                             start=(k == 0), stop=(k == NT - 1),
                             skip_group_check=True)
    pooled = sb.tile([2, N], f32, tag="pooled")
    nc.scalar.copy(pooled[:, :], pb[0:2, 0:N])
    # rep1 = pooled^T W1  (channel-major [64, N])
    pa = ps.tile([128, 2048], f32, tag="pa")
    for (c0, ln) in CHUNKS:
        nc.tensor.matmul(pa[0:HID, c0:c0 + ln], lhsT=cnc(2, CN_W1L1, CN_W1L1 + HID),
                         rhs=pooled[:, c0:c0 + ln], start=True, stop=True)
    bn_block(0, pa, h1m)

    # ---------------- rep2 = h1m @ W2l1 ----------------
    pb = ps.tile([128, 2048], f32, tag="pb")
    for (c0, ln) in CHUNKS:
        nc.tensor.matmul(pb[0:HID, c0:c0 + ln],
                         lhsT=wpc(WP_W2L1, WP_W2L1 + HID),
                         rhs=h1m[:, c0:c0 + ln], start=True, stop=True)
    bn_block(1, pb, h1)

    # ---------------- u2 = h1 @ W1l2 (node-major, bf16) ----------------
    pa = ps.tile([128, 2048], f32, tag="pa")
    for k in range(NT):
        kk = K_LIST[k]
        nc.tensor.matmul(pa[0:kk, HID * k:HID * (k + 1)],
                         lhsT=h1[0:HID, 128 * k:128 * k + kk],
                         rhs=wpc(WP_W1L2, WP_W1L2 + HID),
                         start=True, stop=True)
    nc.scalar.copy(u2[:, :], pa[0:128, 0:NT * HID])

    # ---------------- layer 2: z2 = adj @ u2 ----------------
    pb = ps.tile([128, 2048], f32, tag="pb")
    for k in range(NT):
        kk = K_LIST[k]
        for (c0, ln) in CHUNKS:
            nc.tensor.matmul(pb[0:HID, c0:c0 + ln],
                             lhsT=u2[0:kk, HID * k:HID * (k + 1)],
                             rhs=adjT[0:kk, N * k + c0:N * k + c0 + ln],
                             start=(k == 0), stop=(k == NT - 1),
                             skip_group_check=True)
    bn_block(2, pb, h2m)

    # ---------------- rep2l2 = h2m @ W2l2 ----------------
    pa = ps.tile([128, 2048], f32, tag="pa")
    for (c0, ln) in CHUNKS:
        nc.tensor.matmul(pa[0:HID, c0:c0 + ln],
                         lhsT=wpc(WP_W2L2, WP_W2L2 + HID),
                         rhs=h2m[:, c0:c0 + ln], start=True, stop=True)
    bn_block(3, pa, h2)

    # ---------------- transpose h2 -> node-major ----------------
    pb = ps.tile([128, 2048], f32, tag="pb")
    for k in range(NT):
        kk = K_LIST[k]
        nc.tensor.transpose(pb[0:kk, HID * k:HID * (k + 1)],
                            h2[0:HID, 128 * k:128 * k + kk],
                            cnc(HID, CN_IDENT, CN_IDENT + HID))
    nc.scalar.copy(h2n[:, :], pb[0:128, 0:NT * HID])

    # ---------------- candidates + pooling: C = h2n^T @ paug ----------------
    pa = ps.tile([128, 2048], f32, tag="pa")
    for k in range(NT):
        kk = K_LIST[k]
        nc.tensor.matmul(pa[0:HID, 0:NJ + 1],
                         lhsT=h2n[0:kk, HID * k:HID * (k + 1)],
                         rhs=paug[0:kk, (NJ + 1) * k:(NJ + 1) * (k + 1)],
                         start=(k == 0), stop=(k == NT - 1))
    C = sb.tile([HID, NJ + 1], f32, tag="C")
    nc.scalar.copy(C[:, :], pa[0:HID, 0:NJ + 1])

    # ---------------- actor / critic heads ----------------
    pb2 = ps.tile([128, 2048], f32, tag="pb")
    nc.tensor.matmul(pb2[0:HACT, 0:NJ + 1],
                     lhsT=wpc(WP_ACT, WP_ACT + HACT),
                     rhs=C[:, :], start=True, stop=True)
    nc.tensor.matmul(pb2[0:2 * HACT, 512:513],
                     lhsT=wpc(WP_ACT + HACT, WP_ACT + 3 * HACT),
                     rhs=C[:, NJ:NJ + 1], start=True, stop=True)
    hw = sb.tile([HACT, 2], f32, tag="hw")
    nc.vector.tensor_add(hw[:, 0:1], pb2[0:HACT, 512:513],
                         cnc(HACT, CN_HEADW + 2, CN_HEADW + 3))
    T = sb.tile([HACT, NJ + 1], f32, tag="T")
    nc.scalar.activation(T[:, 0:NJ], pb2[0:HACT, 0:NJ], ACT.Tanh,
                         bias=hw[:, 0:1])
    nc.scalar.activation(T[:, NJ:NJ + 1], pb2[HACT:2 * HACT, 512:513],
                         ACT.Tanh,
                         bias=cnc(HACT, CN_HEADW + 3, CN_HEADW + 4))
    nc.tensor.matmul(pb2[0:1, 1024:1024 + NJ], lhsT=cnc(HACT, CN_HEADW, CN_HEADW + 1),
                     rhs=T[:, 0:NJ], start=True, stop=True)
    nc.tensor.matmul(pb2[0:1, 1536:1537], lhsT=cnc(HACT, CN_HEADW + 1, CN_HEADW + 2),
                     rhs=T[:, NJ:NJ + 1], start=True, stop=True)
    pit = sb.tile([1, NJ + 1], f32, tag="pit")
    sm = sb.tile([1, NJ], f32, tag="sm")
    # scores are tanh-bounded (|s| < ~6), so exp() without max-subtraction
    # is safe; masked entries carry -1e30 -> exp underflows to exactly 0.
    nc.vector.tensor_add(sm[:, :], pb2[0:1, 1024:1024 + NJ],
                         cnc(1, CN_MASK, CN_MASK + NJ))
    red = sb.tile([1, 2], f32, tag="red")
    e = sb.tile([1, NJ], f32, tag="e")
    nc.scalar.activation(e[:, :], sm[:, :], ACT.Exp,
                         bias=cnc(1, CN_ZERO, CN_ZERO + 1),
                         accum_out=red[:, 0:1])
    nc.vector.reciprocal(red[:, 1:2], red[:, 0:1])
    nc.vector.tensor_scalar_mul(pit[:, 0:NJ], e[:, :], red[:, 1:2])
    nc.scalar.activation(pit[:, NJ:NJ + 1], pb2[0:1, 1536:1537],
                         ACT.Copy, bias=float(bc2))
    nc.sync.dma_start(out_e[:, :], pit[:, :])


# ---------------- host side ----------------

def _shuffle_tiles(arr2d, cols):
    """[2000+, cols] node-major -> [128, NT*cols] SBUF tile layout."""
    out = np.zeros((128, NT * cols), dtype=arr2d.dtype)
    for t in range(NT):
        rows = arr2d[128 * t:128 * (t + 1)]
        out[0:rows.shape[0], cols * t:cols * t + cols] = rows
    return out


def _prep_in_maps(x, graph_pool, adj, candidate, mask, params):
    x = np.asarray(x, dtype=np.float32)
    graph_pool = np.asarray(graph_pool, dtype=np.float32)
    adj = np.asarray(adj, dtype=np.float32)
    candidate = np.asarray(candidate).astype(np.int64)
    mask = np.asarray(mask)
    g0, g1 = params['gin'][0], params['gin'][1]
    a, c = params['actor'], params['critic']
    f = lambda t: np.ascontiguousarray(np.asarray(t, dtype=np.float32))

    w1l1 = f(g0['W1'])
    gbe = np.stack([f(g0['g1']), f(g0['be1']), f(g0['g']), f(g0['be']),
                    f(g1['g1']), f(g1['be1']), f(g1['g']), f(g1['be'])],
                   axis=1)  # [64, 8]
    wa1 = f(a['W1'])                       # [128, 32]
    wpack = np.concatenate(
        [f(g0['W2']), f(g1['W1']), f(g1['W2']), gbe,
         np.full((HID, 1), BN_EPS, np.float32),
         wa1[:HID], wa1[HID:], f(c['W1'])], axis=1)
    assert wpack.shape == (HID, WP_COLS)
    headw = np.stack([f(a['W2'])[:, 0], f(c['W2'])[:, 0],
                      np.broadcast_to(f(a['b1']), (HACT,)),
                      np.broadcast_to(f(c['b1']), (HACT,))], axis=1)
    ba2 = float(np.asarray(a['b2']).reshape(-1)[0])
    bc2 = float(np.asarray(c['b2']).reshape(-1)[0])

    in_maps = []
    for b in range(B):
        sl = slice(b * N, (b + 1) * N)
        adjT = np.ascontiguousarray(adj[sl, sl].T)
        adjT = _shuffle_tiles(adjT, N).astype(ml_dtypes.bfloat16)
        xn = _shuffle_tiles(x[sl], 2).astype(ml_dtypes.bfloat16)
        ph = np.zeros((N, NJ + 1), dtype=np.float32)
        ph[candidate[b], np.arange(NJ)] = 1.0
        ph[:, NJ] = graph_pool[b, sl]
        paug = np.zeros((128, NT * (NJ + 1) + WP_COLS), dtype=np.float32)
        paug[:, :NT * (NJ + 1)] = _shuffle_tiles(ph, NJ + 1)
        paug[0:HID, NT * (NJ + 1):] = wpack
        maskval = (np.where(mask[b], NEG_BIG, 0.0) + ba2) \
            .astype(np.float32).reshape(1, NJ)
        consts = np.zeros((128, CN_COLS), dtype=np.float32)
        consts[0:2, CN_W1L1:CN_W1L1 + HID] = w1l1
        consts[0:HACT, CN_HEADW:CN_HEADW + 4] = headw
        consts[0:1, CN_MASK:CN_MASK + NJ] = maskval
        consts[0:HID, CN_IDENT:CN_IDENT + HID] = np.eye(HID, dtype=np.float32)
        in_maps.append({
            "adjT": adjT, "xn": xn, "paug": paug, "consts": consts,
        })
    return in_maps, bc2


def run(inputs: dict, reps: int = 1, nc=None):
    """Run on hardware; returns ((pi, v), nc) so callers can reuse the graph."""
    in_maps, bc2 = _prep_in_maps(
        inputs['x'], inputs['graph_pool'], inputs['adj'],
        inputs['candidate'], inputs['mask'], inputs['params'])
    if nc is None:
        nc = build_graph(bc2, reps=reps)
    res = run_bass_kernel_spmd(nc, in_maps, core_ids=list(range(B)))
    outs = [res.results[i]["out"] for i in range(B)]
    pi = np.stack([o[0, :NJ] for o in outs]).astype(np.float32)[:, :, None]
    v = np.stack([o[0, NJ:NJ + 1] for o in outs]).astype(np.float32)
    return (pi, v), nc


def kernel(x, graph_pool, padded_nei, adj, candidate, mask, params):
    (pi, v), _ = run({'x': x, 'graph_pool': graph_pool, 'adj': adj,
                      'candidate': candidate, 'mask': mask, 'params': params})
    return pi, v
